# revision 3
# baseline (speedup 1.0000x reference)
"""Trainium2 Bass kernel for the LTC (liquid time-constant) memory cell.

Model (see reference): v-state recurrence over T=128 timesteps, each with 6
ODE unfold iterations:
    v' = (cm_t*v + gl*vl + num_syn) / (cm_t + gl + den_syn + eps)
with 2 recurrent synapses per neuron (self: u, pair: (u+dim)%U) and one
sensory synapse (source d = u%dim).

Sharding: 8 cores; core c owns the 128 neuron *pairs* {u=c*128+p, u+1024}
for p in [0,128), with the FULL batch B=32. Each partition p holds one
pair; state is one [128, 64] tile (cols 0:32 = half A batch, 32:64 = half
B batch), carried as w = v + 1 so the erev=-1 numerator collapses to
q = cmt*w + nd (no recurrent term).

Engine split per unfold:
 - ACT: 4 sigmoids (per-partition scale/bias fuse the affine),
 - DVE: d1/den scalar_tensor_tensor chains per half + ONE fused
   approximate-divide custom op w' = q/den on the [128,64] tile
   (quadratic-seed bit-trick reciprocal, ~8e-5 rel err, 8 ALU stages),
 - Pool(GpSimd): q = w*cmt + nd as two tensor_tensor ops with
   broadcast param tiles; also the per-timestep sensory ds/nd tiles.
Input x is preloaded to SBUF once; input affine folds into the sensory
ACT scale/bias on the host; output affine applied on the host.
"""

import numpy as np

import concourse.bacc as bacc
import concourse.mybir as mybir
from concourse import tile
from concourse.bass_utils import run_bass_kernel_spmd

ODE_UNFOLDS = 6
EPS = 1e-8
B = 32
T = 128
DIM = 1024
U = 2 * DIM
NCORES = 8
P = 128  # partitions = pairs per core

F32 = mybir.dt.float32
AF = mybir.ActivationFunctionType
OP = mybir.AluOpType

# ---------------------------------------------------------------------------
# Custom DVE op: fused approximate divide  out = in1 / in0
# n = bitcast(~x); m = x*n lands in [-4.5,-4] for all finite x>0; a quadratic
# Chebyshev fit of 1/m then out = n*poly(m)*in1. ~8.3e-5 max rel err.
DIV_CONSTS = {"s0": -0.7066511871005156, "s1": -0.16633655560380672,
              "imm2": -0.013042133349701725}


def _ref_div(in0, in1, c0, c1, c2):
    x = in0.astype(np.float32)
    n = (~x.view(np.int32)).view(np.float32)
    m = (x * n).astype(np.float32)
    t = (np.float32(c2) * m).astype(np.float32)
    t = (t + np.float32(c1)).astype(np.float32)
    t = (t * m).astype(np.float32)
    t = (t + np.float32(c0)).astype(np.float32)
    r = (t * n).astype(np.float32)
    return (r * in1).astype(np.float32)


def _make_div_op():
    import concourse.dve_ops as dve_ops
    from concourse.dve_spec import (Spec, Src0, Src1, Bin, AluOp, lower,
                                    _has_src1)
    from concourse.dve_uop import DveOpSpec

    name = "TENSOR_DIV_APPROX_ANT"
    for o in dve_ops.OPS:
        if o.name == name:
            return o
    from concourse.dve_spec import C0, C1, C2
    _n = Bin(AluOp.BITWISE_NOT, Src0, Src0)
    _m = Src0 * _n
    body = ((C2 * _m + C1) * _m + C0) * _n * Src1
    spec = Spec(body=body, reference=_ref_div)
    row = max(dve_ops._SUB_OPCODE_FOR_NAME.values()) + 1
    assert row < 0x20
    dve_ops._SUB_OPCODE_FOR_NAME[name] = row
    shas = {}
    for ver in ("v3", "v4"):
        compiled = DveOpSpec(name=name, opcode=row, uops=lower(spec, ver=ver),
                             rd1_en=_has_src1(spec))
        shas[ver] = compiled.sha(ver)
    op = dve_ops.DveOp(name, spec, subdim=False, uops_sha=shas)
    dve_ops.OPS.append(op)
    dve_ops.CUSTOM_DVE_SPECS[name] = spec
    return op


# pp column indices (per half; half B adds NPARAM)
(C_SIG0, C_B0P, C_SIG1, C_B1P, C_W0, C_W1, C_SSIG, C_NSMS) = range(8)
NPARAM = 8
# pconst [P, 5*64] broadcast tiles: cmt, spsw, gcme, wps, ggp
(K_CMT, K_SPSW, K_GCME, K_WPS, K_GGP) = range(5)
NCONST = 5


def _softplus(x):
    x = x.astype(np.float64)
    return np.log1p(np.exp(-np.abs(x))) + np.maximum(x, 0.0)


def _build_v2(wbufs=4):
    div_op = _make_div_op()
    nc = bacc.Bacc(trn_type="TRN2")
    xin_d = nc.dram_tensor("xin", [P, T * B], F32, kind="ExternalInput")
    pp_d = nc.dram_tensor("pp", [P, 2 * NPARAM], F32, kind="ExternalInput")
    pc_d = nc.dram_tensor("pc", [P, NCONST * 64], F32, kind="ExternalInput")
    out_d = nc.dram_tensor("out", [P, B], F32, kind="ExternalOutput")

    with tile.TileContext(nc) as tc:
        with tc.tile_pool(name="const", bufs=1) as cpool, \
             tc.tile_pool(name="work", bufs=wbufs) as wpool:
            xin = cpool.tile([P, T * B], F32, tag="xin", name="xin_t")
            pp = cpool.tile([P, 2 * NPARAM], F32, tag="pp", name="pp_t")
            pc = cpool.tile([P, NCONST * 64], F32, tag="pc", name="pc_t")
            nc.sync.dma_start(xin[:], xin_d[:])
            nc.sync.dma_start(pp[:], pp_d[:])
            nc.sync.dma_start(pc[:], pc_d[:])

            def par(h, c):  # per-partition scalar AP for half h param c
                j = h * NPARAM + c
                return pp[:, j:j + 1]

            def kc(k):  # [P,64] broadcast const tile slice
                return pc[:, k * 64:(k + 1) * 64]

            # state (w = v+1), ping-pong [P,64]
            w = [cpool.tile([P, 64], F32, tag=f"w{i}", name=f"w{i}")
                 for i in range(2)]
            nc.vector.memset(w[0][:], 1.0)
            # sensory tiles, ping-pong per timestep
            sg = [cpool.tile([P, 64], F32, tag=f"sg{i}", name=f"sg{i}")
                  for i in range(2)]
            ds = [cpool.tile([P, 64], F32, tag=f"ds{i}", name=f"ds{i}")
                  for i in range(2)]
            nd = [cpool.tile([P, 64], F32, tag=f"nd{i}", name=f"nd{i}")
                  for i in range(2)]

            def wtile(tag, n=32):
                return wpool.tile([P, n], F32, tag=tag, name=tag)

            def sens_sig(t, pi):
                """sensory sigmoids for timestep t into sg[pi] slices."""
                xt = xin[:, t * B:(t + 1) * B]
                nc.scalar.activation(sg[pi][:, 0:32], xt, AF.Sigmoid,
                                     bias=par(0, C_NSMS),
                                     scale=par(0, C_SSIG))
                nc.scalar.activation(sg[pi][:, 32:64], xt, AF.Sigmoid,
                                     bias=par(1, C_NSMS),
                                     scale=par(1, C_SSIG))

            def sens_ds(pi):
                tmp = wtile("dsm", 64)
                nc.gpsimd.tensor_tensor(tmp[:], sg[pi][:], kc(K_SPSW),
                                        OP.mult)
                nc.gpsimd.tensor_tensor(ds[pi][:], tmp[:], kc(K_GCME),
                                        OP.add)

            def sens_nd(pi):
                tmp = wtile("ndm", 64)
                nc.gpsimd.tensor_tensor(tmp[:], sg[pi][:], kc(K_WPS),
                                        OP.mult)
                nc.gpsimd.tensor_tensor(nd[pi][:], tmp[:], kc(K_GGP),
                                        OP.add)

            # prologue: sensory for t=0
            sens_sig(0, 0)
            sens_ds(0)
            sens_nd(0)

            cur = 0
            for t in range(T):
                pi = t % 2
                more = t + 1 < T
                for k in range(ODE_UNFOLDS):
                    wc = w[cur][:]
                    wA = w[cur][:, 0:32]
                    wB = w[cur][:, 32:64]
                    # q = w*cmt + nd on Pool (off critical path)
                    qm = wtile("qm", 64)
                    q = wtile("q", 64)
                    nc.gpsimd.tensor_tensor(qm[:], wc, kc(K_CMT), OP.mult)
                    nc.gpsimd.tensor_tensor(q[:], qm[:], nd[pi][:], OP.add)
                    # 4 sigmoids on ACT
                    s0A = wtile("s0A")
                    s1A = wtile("s1A")
                    s0B = wtile("s0B")
                    s1B = wtile("s1B")
                    nc.scalar.activation(s0A[:], wA, AF.Sigmoid,
                                         bias=par(0, C_B0P),
                                         scale=par(0, C_SIG0))
                    nc.scalar.activation(s1A[:], wB, AF.Sigmoid,
                                         bias=par(0, C_B1P),
                                         scale=par(0, C_SIG1))
                    nc.scalar.activation(s0B[:], wB, AF.Sigmoid,
                                         bias=par(1, C_B0P),
                                         scale=par(1, C_SIG0))
                    nc.scalar.activation(s1B[:], wA, AF.Sigmoid,
                                         bias=par(1, C_B1P),
                                         scale=par(1, C_SIG1))
                    # mid-timestep sensory for t+1 (ACT/Pool slack)
                    if more and k == 2:
                        sens_sig(t + 1, 1 - pi)
                    if more and k == 3:
                        sens_ds(1 - pi)
                    if more and k == 4:
                        sens_nd(1 - pi)
                    # den chains on DVE
                    d1A = wtile("d1A")
                    d1B = wtile("d1B")
                    den = wtile("den", 64)
                    nc.vector.scalar_tensor_tensor(
                        d1A[:], s0A[:], par(0, C_W0), ds[pi][:, 0:32],
                        OP.mult, OP.add)
                    nc.vector.scalar_tensor_tensor(
                        den[:, 0:32], s1A[:], par(0, C_W1), d1A[:],
                        OP.mult, OP.add)
                    nc.vector.scalar_tensor_tensor(
                        d1B[:], s0B[:], par(1, C_W0), ds[pi][:, 32:64],
                        OP.mult, OP.add)
                    nc.vector.scalar_tensor_tensor(
                        den[:, 32:64], s1B[:], par(1, C_W1), d1B[:],
                        OP.mult, OP.add)
                    # fused divide: w' = q / den
                    nxt = 1 - cur
                    nc.vector._custom_dve(
                        div_op, out=w[nxt][:], in0=den[:], in1=q[:],
                        s0=DIV_CONSTS["s0"], s1=DIV_CONSTS["s1"],
                        imm2=DIV_CONSTS["imm2"])
                    cur = nxt

            nc.sync.dma_start(out_d[:], w[cur][:, 0:32])
    nc.compile()
    return nc


_NC_CACHE = {}


def _get_nc():
    if "v2" not in _NC_CACHE:
        _NC_CACHE["v2"] = _build_v2()
    return _NC_CACHE["v2"]


def _host_params(c, gleak, vleak, cm, w, sigma, mu, erev,
                 sens_w, sens_sigma, sens_mu, sens_erev,
                 input_w, input_b):
    """pp [128, 2*NPARAM] and pc [128, NCONST*64] for core c."""
    d = c * P + np.arange(P)
    pp = np.zeros((P, 2 * NPARAM), np.float32)
    pcn = np.zeros((P, NCONST, 2, 32), np.float32)
    for h in range(2):
        u = h * DIM + d
        sp_w = _softplus(w[u])
        sp_gl = _softplus(gleak[u])
        sp_sw = _softplus(sens_w[u])
        cmt = _softplus(cm[u]) * ODE_UNFOLDS
        o = h * NPARAM
        # state shift w = v + 1: sigmoid biases absorb -sigma
        pp[:, o + C_SIG0] = sigma[u, 0]
        pp[:, o + C_B0P] = -(mu[u, 0] + 1.0) * sigma[u, 0]
        pp[:, o + C_SIG1] = sigma[u, 1]
        pp[:, o + C_B1P] = -(mu[u, 1] + 1.0) * sigma[u, 1]
        pp[:, o + C_W0] = sp_w[:, 0]
        pp[:, o + C_W1] = sp_w[:, 1]
        pp[:, o + C_SSIG] = sens_sigma[u] * input_w[d]
        pp[:, o + C_NSMS] = (input_b[d] - sens_mu[u]) * sens_sigma[u]
        gcme = cmt + sp_gl + EPS
        pcn[:, K_CMT, h, :] = cmt[:, None]
        pcn[:, K_SPSW, h, :] = sp_sw[:, None]
        pcn[:, K_GCME, h, :] = gcme[:, None]
        pcn[:, K_WPS, h, :] = (sp_sw * (1.0 + sens_erev[u]))[:, None]
        pcn[:, K_GGP, h, :] = (sp_gl * vleak[u] + sp_gl + EPS)[:, None]
    return pp, pcn.reshape(P, NCONST * 64).astype(np.float32)


def kernel(inputs, gleak, vleak, cm, w, sigma, mu, erev,
           sens_w, sens_sigma, sens_mu, sens_erev,
           input_w, input_b, output_w, output_b, _trace=False):
    inputs = np.asarray(inputs, np.float32)
    args = dict(gleak=np.asarray(gleak, np.float32),
                vleak=np.asarray(vleak, np.float32),
                cm=np.asarray(cm, np.float32),
                w=np.asarray(w, np.float32),
                sigma=np.asarray(sigma, np.float32),
                mu=np.asarray(mu, np.float32),
                erev=np.asarray(erev, np.float32),
                sens_w=np.asarray(sens_w, np.float32),
                sens_sigma=np.asarray(sens_sigma, np.float32),
                sens_mu=np.asarray(sens_mu, np.float32),
                sens_erev=np.asarray(sens_erev, np.float32),
                input_w=np.asarray(input_w, np.float32),
                input_b=np.asarray(input_b, np.float32))
    # the fused numerator (q = cmt*w + nd) relies on erev == -1 exactly
    assert np.allclose(args["erev"], -1.0), "kernel requires erev == -1"

    in_maps = []
    for c in range(NCORES):
        xc = inputs[:, :, c * P:(c + 1) * P]          # [B,T,P]
        xin = np.ascontiguousarray(
            xc.transpose(2, 1, 0).reshape(P, T * B))  # [P, t*B+b]
        pp, pcn = _host_params(c, **args)
        in_maps.append({"xin": xin, "pp": pp, "pc": pcn})

    nc = _get_nc()
    res = run_bass_kernel_spmd(nc, in_maps, core_ids=list(range(NCORES)),
                               trace=_trace)

    out = np.zeros((B, DIM), np.float32)
    for c in range(NCORES):
        out[:, c * P:(c + 1) * P] = res.results[c]["out"].T
    out = out - 1.0  # state was carried as w = v + 1
    out = out * np.asarray(output_w, np.float32) + np.asarray(output_b, np.float32)
    if _trace:
        kernel.last_results = res
    return out
